# revision 17
# baseline (speedup 1.0000x reference)
"""Trainium2 Bass kernel for nn_Attention_layer (per-label MLP attention).

Computes, for full inputs:
    h = relu(cat(label_emb, unlabel_emb) @ W1 + b1)        [N, B, H]
    scores = h @ W2 + b2                                   [N, B]
    out = softmax(scores.T * dis_lab, axis=1)              [B, N]

Distribution: pure data-parallel over batch B across 8 NeuronCores
(B=1024 -> 128 rows/core). No collectives; softmax is over the station
axis N which stays local to a core.

Host prep: W2 is folded into W1 (W1' = W1 * w2 per column) and the whole
W1' is scaled by 2^13 (descale is folded into dis_lab host-side, so the
device never descales). Columns are ordered
    [bf16-pos | fp8-pos | fp8-neg | bf16-neg]
where the 512 largest-|w2| columns stay bf16 and the rest are fp8e4
(the 2^13 scale keeps them in fp8's normal range). The label embeddings
ship twice: bf16 (for the bf16 columns' matmul) and fp8.

Device (per core): per station the PE computes the label matmul - bf16
ranges as two K=128 matmuls, fp8 ranges as one DoubleRow K=256 matmul at
2x rate - and injects the precomputed unlabel contribution U' into the
positive (= ACT's) columns via an identity matmul. Measured HW: any
ACT/DVE elementwise op costs ~170ns fixed + ~1.04ns/col, so the
relu+score reduction is split once:
  - ACT: relu+accum over all positive columns [0:jpos) (injected).
  - DVE: one scalar_tensor_tensor over all negative columns [jpos:H):
    min(psum, -U') + accum, using min(a+b,0) = b + min(a,-b); the
    constant C' = sum_{j>=jpos} U'_j is added in the tail.
Tail: scores * dis_lab(prescaled), softmax over the 64 stations
(no max-subtraction: |scores*dis| < 6 so exp is safe in f32).
"""

import os
import sys

for _p in (
    "/root/.axon_site",
    "/root/.axon_site/_ro/trn_rl_repo",
    "/root/.axon_site/_ro/pypackages",
):
    if _p not in sys.path and os.path.isdir(_p):
        sys.path.append(_p)

import ml_dtypes
import numpy as np

import concourse.bass as bass
import concourse.mybir as mybir
import concourse.tile as tile
from concourse import bacc
from concourse.bass_utils import run_bass_kernel_spmd
from concourse.masks import make_identity

N, B, EMB, UEMB, H = 64, 1024, 256, 256, 1024
N_CORES = 8
BS = B // N_CORES  # 128 batch rows per core
KL = EMB // 128  # label-emb contraction chunks
KU = UEMB // 128  # unlabel-emb contraction chunks
F32 = mybir.dt.float32
BF16 = mybir.dt.bfloat16
F8 = mybir.dt.float8e4

NBIG = 256          # columns kept in bf16 (largest |w2|)
NF8 = H - NBIG      # fp8 columns
SCALE_BITS = 13     # global power-of-2 scale folded into W1'

PROFILE = False
LAST_EXEC_NS = None
TRACE_DIR = None

_cache = {}


def _build(jpos, f0, f1, b2val, zero_b1):
    # column layout: [0:f0) bf16-pos, [f0:jpos) fp8-pos, [jpos:f1) fp8-neg,
    # [f1:H) bf16-neg
    nc = bacc.Bacc("TRN2", target_bir_lowering=False, debug=False,
                   num_devices=N_CORES)
    xlabT = nc.dram_tensor("xlabT", [KL, 128, N, BS], BF16,
                           kind="ExternalInput").ap()
    xlab8 = nc.dram_tensor("xlab8", [128, KL, N, BS], F8,
                           kind="ExternalInput").ap()
    xunlT = nc.dram_tensor("xunlT", [KU, 128, BS], BF16,
                           kind="ExternalInput").ap()
    dis = nc.dram_tensor("dis", [BS, N], F32, kind="ExternalInput").ap()
    w1b = nc.dram_tensor("w1b", [128, 2, NBIG], BF16,
                         kind="ExternalInput").ap()
    w18 = nc.dram_tensor("w18", [128, 2, NF8], F8,
                         kind="ExternalInput").ap()
    w1u = nc.dram_tensor("w1u", [2, 128, H], BF16,
                         kind="ExternalInput").ap()
    b1p = nc.dram_tensor("b1p", [H], F32, kind="ExternalInput").ap()
    out = nc.dram_tensor("out", [BS, N], F32, kind="ExternalOutput").ap()

    with tile.TileContext(nc) as tc:
        _emit(tc, out, xlabT, xlab8, xunlT, dis, w1b, w18, w1u, b1p,
              jpos, f0, f1, b2val, zero_b1)
    nc.compile()
    return nc


def _emit(tc, out, xlabT_d, xlab8_d, xunlT_d, dis, w1b_d, w18_d, w1u_d,
          b1p, jpos, f0, f1, b2val, zero_b1):
    nc = tc.nc
    AF = mybir.ActivationFunctionType
    ALU = mybir.AluOpType
    PM = mybir.MatmulPerfMode
    nb1 = f0            # bf16 cols in bank0
    nb2 = H - f1        # bf16 cols in bank1
    assert jpos <= 512 <= f1

    with tc.tile_pool(name="consts", bufs=1) as consts:
        # --- warmup weights: memset, no on-device generation needed ---
        warmw = consts.tile([128, 128], BF16, tag="warmw")
        nc.vector.memset(warmw, 0.0)
        ident = consts.tile([128, 128], BF16, tag="ident")
        make_identity(nc, ident)

        w1b = consts.tile([128, 2, NBIG], BF16, tag="w1b")
        w18 = consts.tile([128, 2, NF8], F8, tag="w18")
        w1u = []
        for k in range(KU):
            t = consts.tile([128, H], BF16, tag=f"w1u_{k}")
            w1u.append(t)
        xunlT = []
        for k in range(KU):
            t = consts.tile([128, 128], BF16, tag=f"xunlT_{k}")
            nc.gpsimd.dma_start(out=t, in_=xunlT_d[k])
            xunlT.append(t)
        nc.scalar.dma_start(out=w1u[0], in_=w1u_d[0])
        nc.scalar.dma_start(out=w1u[1], in_=w1u_d[1])

        xlabT = consts.tile([128, KL, N, 128], BF16, tag="xlabT")
        xlab8 = consts.tile([128, KL, N, 128], F8, tag="xlab8")
        GRP = 8

        # SP queue: bf16 weights + bf16 label chunks (PE consumes first)
        nc.sync.dma_start(out=w1b, in_=w1b_d)
        for k in range(KL):
            nc.sync.dma_start(out=xlabT[:, k, 0:4, :],
                              in_=xlabT_d[k, :, 0:4, :])
        # ACT queue: fp8 weights + first fp8 label chunks (later chunks
        # are issued from inside the main loop to not block station 0)
        nc.scalar.dma_start(out=w18, in_=w18_d)
        nc.scalar.dma_start(out=xlab8[:, :, 0:4, :],
                            in_=xlab8_d[:, :, 0:4, :])
        for k in range(KL):
            nc.sync.dma_start(out=xlabT[:, k, 4:8, :],
                              in_=xlabT_d[k, :, 4:8, :])
        nc.scalar.dma_start(out=xlab8[:, :, 4:8, :],
                            in_=xlab8_d[:, :, 4:8, :])
        for g in range(8, N, 2 * GRP):
            ge = min(g + 2 * GRP, N)
            for k in range(KL):
                nc.sync.dma_start(
                    out=xlabT[:, k, g:ge, :],
                    in_=xlabT_d[k, :, g:ge, :])

        def emit_x8_dma(g):
            ge = min(g + 2 * GRP, N)
            nc.scalar.dma_start(out=xlab8[:, :, g:ge, :],
                                in_=xlab8_d[:, :, g:ge, :])

        dis_sb = consts.tile([128, N], F32, tag="dis")
        nc.gpsimd.dma_start(out=dis_sb, in_=dis)
        if not zero_b1:
            b1bc = consts.tile([128, H], F32, tag="b1bc")
            b1_bcast = bass.AP(tensor=b1p.tensor, offset=b1p.offset,
                               ap=[[0, 128]] + list(b1p.ap))
            nc.scalar.dma_start(out=b1bc, in_=b1_bcast)

        # --- unlabel branch: U' = unl_emb @ W1_unl' (+ b1'), scaled ---
        posU = consts.tile([128, H], BF16, tag="posU")
        negU = consts.tile([128, H], BF16, tag="negU")
        Cap = consts.tile([128, 1], F32, tag="C")
        with tc.tile_pool(name="pre_psum", bufs=1, space="PSUM") as pre_psum:
            # PE warmup: dummy matmuls while input DMAs stream; rides out
            # the HAM activity throttle window.
            warm = pre_psum.tile([128, 128], F32, tag="warm")
            NWARM = 4
            for w in range(NWARM):
                nc.tensor.matmul(warm, warmw, warmw,
                                 start=(w == 0), stop=(w == NWARM - 1))
            psu = pre_psum.tile([128, H], F32, tag="psu")
            for half in range(2):
                hs = slice(512 * half, 512 * (half + 1))
                for k in range(KU):
                    nc.tensor.matmul(psu[:, hs], xunlT[k], w1u[k][:, hs],
                                     start=(k == 0), stop=(k == KU - 1))
            if not zero_b1:
                nc.vector.tensor_tensor(out=psu, in0=psu, in1=b1bc,
                                        op=ALU.add)
            nc.vector.tensor_copy(posU[:, :jpos], psu[:, :jpos])
            nc.vector.tensor_scalar_mul(negU[:, jpos:], psu[:, jpos:], -1.0)
            # C' from the rounded negU so clamp and offset agree exactly.
            nc.vector.reduce_sum(Cap, negU[:, jpos:],
                                 axis=mybir.AxisListType.X, negate=True)

        # --- per-station score accumulators ---
        sA = consts.tile([128, N], F32, tag="sA")
        sM = consts.tile([128, N], F32, tag="sM")

        # --- main loop ---
        with tc.tile_pool(name="psum", bufs=4, space="PSUM") as psum_pool, \
             tc.tile_pool(name="relu_sb", bufs=3) as relu_pool:
            for n in range(N):
                if n < 4 and n * 16 + 8 < N:
                    emit_x8_dma(n * 16 + 8)
                ph = psum_pool.tile([128, H], F32, tag="ph", name=f"ph_{n}")
                lab16 = [xlabT[:, k, n, :] for k in range(KL)]
                lab8 = xlab8[:, :, n, :]
                # bank0 group: bf16 [0:f0), fp8 DR [f0:512), inject [0:jpos)
                for k in range(KL):
                    nc.tensor.matmul(ph[:, :f0], lab16[k], w1b[:, k, :nb1],
                                     start=(k == 0), stop=False)
                nc.tensor.matmul(ph[:, f0:512], lab8, w18[:, :, :512 - f0],
                                 start=False, stop=False,
                                 perf_mode=PM.DoubleRow)
                nc.tensor.matmul(ph[:, :jpos], ident, posU[:, :jpos],
                                 start=False, stop=True)
                # bank1 group: fp8 DR [512:f1), bf16 [f1:H)
                nc.tensor.matmul(ph[:, 512:f1], lab8, w18[:, :, 512 - f0:],
                                 start=True, stop=False,
                                 perf_mode=PM.DoubleRow)
                for k in range(KL):
                    nc.tensor.matmul(ph[:, f1:], lab16[k], w1b[:, k, nb1:],
                                     start=False, stop=(k == KL - 1))

                rl = relu_pool.tile([128, H], BF16, tag="rl")
                nc.scalar.activation(out=rl[:, :jpos], in_=ph[:, :jpos],
                                     func=AF.Relu, accum_out=sA[:, n:n + 1])
                nc.vector.scalar_tensor_tensor(
                    rl[:, jpos:], ph[:, jpos:], 1.0, negU[:, jpos:],
                    op0=ALU.mult, op1=ALU.min,
                    accum_out=sM[:, n:n + 1])

        # --- scores assembly + softmax tail (all [128, N] sized) ---
        _emit_tail(tc, consts, out, dis_sb, sA, sM, Cap, b2val)


def _emit_tail(tc, consts, out, dis_sb, sA, sM, Cap, b2val):
    nc = tc.nc
    AF = mybir.ActivationFunctionType
    ALU = mybir.AluOpType
    t1 = consts.tile([128, N], F32, tag="t1")
    # t1 = (sA + C') + sM in one pass
    nc.vector.scalar_tensor_tensor(t1, sA, Cap, sM, op0=ALU.add, op1=ALU.add)
    if b2val != 0.0:
        # accumulators are 2^SCALE_BITS-scaled; b2 must match
        nc.vector.tensor_scalar_add(t1, t1, float(b2val * 2.0 ** SCALE_BITS))
    att = consts.tile([128, N], F32, tag="att")
    # dis_sb is prescaled by 2^-SCALE_BITS host-side: descale happens here
    nc.vector.tensor_tensor(out=att, in0=t1, in1=dis_sb, op=ALU.mult)

    # |scores * dis| < ~6, exp() is safe in f32 without max-subtraction
    ex = consts.tile([128, N], F32, tag="ex")
    sume = consts.tile([128, 1], F32, tag="sume")
    nc.scalar.activation(out=ex, in_=att, func=AF.Exp, accum_out=sume)
    rs = consts.tile([128, 1], F32, tag="rs")
    nc.vector.reciprocal(rs, sume)
    res = consts.tile([128, N], F32, tag="res")
    nc.vector.tensor_scalar_mul(res, ex, rs)
    nc.sync.dma_start(out=out[:64, :], in_=res[:64, :])
    nc.scalar.dma_start(out=out[64:, :], in_=res[64:, :])


def kernel(unlabel_emb, label_emb, dis_lab, W1, b1, W2, b2):
    global LAST_EXEC_NS, TRACE_DIR
    unlabel_emb = np.asarray(unlabel_emb, dtype=np.float32)
    label_emb = np.asarray(label_emb, dtype=np.float32)
    dis_lab = np.asarray(dis_lab, dtype=np.float32)
    W1 = np.asarray(W1, dtype=np.float32)
    b1 = np.asarray(b1, dtype=np.float32)
    W2 = np.asarray(W2, dtype=np.float32)
    b2 = np.asarray(b2, dtype=np.float32)

    # Fold W2 into W1 columns; order [bf16-pos | fp8-pos | fp8-neg |
    # bf16-neg] with the NBIG largest-|w2| columns in bf16.
    w2 = W2[:, 0]
    pos = w2 > 0
    order = np.argsort(-np.abs(w2), kind="stable")
    big = np.zeros(H, bool)
    big[order[:NBIG]] = True
    g0 = np.where(big & pos)[0]
    g1 = np.where(~big & pos)[0]
    g2 = np.where(~big & ~pos)[0]
    g3 = np.where(big & ~pos)[0]
    perm = np.concatenate([g0, g1, g2, g3])
    jpos = len(g0) + len(g1)
    f0 = len(g0)
    f1 = jpos + len(g2)
    S = 2.0 ** SCALE_BITS

    W1f = (W1 * w2[None, :])[:, perm] * S
    b1f = (b1 * w2)[perm] * S
    b2val = float(b2[0])

    zero_b1 = not np.any(b1f)
    key = (jpos, f0, f1, b2val, zero_b1)
    if key not in _cache:
        _cache[key] = _build(jpos, f0, f1, b2val, zero_b1)
    nc = _cache[key]

    lab_cols = np.concatenate([np.arange(f0), np.arange(f1, H)])
    f8_cols = np.arange(f0, f1)
    # w1b[p, kk, jj] = W1f[kk*128+p, lab_cols[jj]]  (label rows 0:256)
    w1b_np = np.ascontiguousarray(
        W1f[:EMB, lab_cols].reshape(2, 128, NBIG).transpose(1, 0, 2)
    ).astype(ml_dtypes.bfloat16)
    w18_np = np.ascontiguousarray(
        W1f[:EMB, f8_cols].reshape(2, 128, NF8).transpose(1, 0, 2)
    ).astype(ml_dtypes.float8_e4m3fn)
    w1u_np = np.ascontiguousarray(
        W1f[EMB:].reshape(2, 128, H)).astype(ml_dtypes.bfloat16)
    b1p_np = b1f.astype(np.float32)
    dis_scaled = np.ascontiguousarray(dis_lab / S)

    in_maps = []
    for c in range(N_CORES):
        sh = slice(c * BS, (c + 1) * BS)
        # [N, BS, EMB] -> [EMB, N, BS]
        lab_t = np.ascontiguousarray(
            label_emb[:, sh, :].transpose(2, 0, 1))
        lab8_t = np.ascontiguousarray(
            lab_t.reshape(2, 128, N, BS).transpose(1, 0, 2, 3))
        unl_t = np.ascontiguousarray(
            unlabel_emb[sh].T).reshape(KU, 128, BS)
        in_maps.append({
            "xlabT": lab_t.reshape(KL, 128, N, BS).astype(ml_dtypes.bfloat16),
            "xlab8": lab8_t.astype(ml_dtypes.float8_e4m3fn),
            "xunlT": unl_t.astype(ml_dtypes.bfloat16),
            "dis": dis_scaled[sh],
            "w1b": w1b_np,
            "w18": w18_np,
            "w1u": w1u_np,
            "b1p": b1p_np,
        })

    kwargs = {}
    if PROFILE:
        try:
            import ntff_shim  # noqa: F401  (registers the axon NTFF hook)
        except ImportError:
            pass
        import tempfile
        TRACE_DIR = tempfile.mkdtemp(prefix="bass_trace_")
        kwargs = dict(trace=True, tmpdir=TRACE_DIR)
    res = run_bass_kernel_spmd(nc, in_maps, core_ids=list(range(N_CORES)),
                               **kwargs)
    if PROFILE:
        LAST_EXEC_NS = res.exec_time_ns
    return np.concatenate([res.results[c]["out"] for c in range(N_CORES)],
                          axis=0)
